# revision 3
# baseline (speedup 1.0000x reference)
"""Block-diagonal complex matmul kernel for trn2 (8 NeuronCores).

Reference computation:
  xp = take(x, perm_idx, axis=-2).reshape(B, 2, M, S)
  y_re = xp_re @ hr1 + xp_im @ hi1   (per block a of M)
  y_im = xp_re @ hi2 + xp_im @ hr2
  out  = stack([y_re, y_im], 1).reshape(B, 2, N, R)

Sharding: block dim M=1024 split across 8 cores (128 blocks each).
Permutation gather + all layout shuffles happen host-side in numpy.

Per-core device kernel, per block a:
  psum[16, 256] = xT_re[:, a] .T @ [hr1[a] | hi2[a]]   (start)
                + xT_im[:, a] .T @ [hi1[a] | hr2[a]]   (stop)
  -> cols 0:128 = y_re[a], cols 128:256 = y_im[a]
"""

import os
import numpy as np

B = 16
N = 4096
R = 32
M = 1024   # blocks
S = 128    # block size (contract dim)
NCORES = 8
MLOC = M // NCORES   # 128 blocks per core
NB = 8               # blocks per weight DMA group (2 MiB per dma_start)
NGRP = MLOC // NB

_NC_CACHE = {}


def _build_nc():
    import concourse.bacc as bacc
    import concourse.bass as bass
    import concourse.mybir as mybir
    from concourse import tile

    mm_dt = mybir.dt.float32
    nc = bacc.Bacc(None, target_bir_lowering=False)

    xr = nc.dram_tensor("xr", [S, MLOC * B], mm_dt, kind="ExternalInput")
    xi = nc.dram_tensor("xi", [S, MLOC * B], mm_dt, kind="ExternalInput")
    w = nc.dram_tensor("w", [S, MLOC * 4 * S], mm_dt, kind="ExternalInput")
    y = nc.dram_tensor("y", [B, MLOC * 2 * S], mybir.dt.float32, kind="ExternalOutput")

    with tile.TileContext(nc) as tc:
        with (
            tc.tile_pool(name="xp", bufs=1) as xpool,
            tc.tile_pool(name="wp", bufs=3) as wpool,
            tc.tile_pool(name="op", bufs=3) as opool,
            tc.tile_pool(name="ps", bufs=8, space=bass.MemorySpace.PSUM) as ps,
        ):
            xr_t = xpool.tile([S, MLOC * B], mm_dt)
            xi_t = xpool.tile([S, MLOC * B], mm_dt)
            nc.sync.dma_start(xr_t[:], xr[:])
            nc.sync.dma_start(xi_t[:], xi[:])
            for g in range(NGRP):
                wt = wpool.tile([S, NB * 4 * S], mm_dt)
                nc.sync.dma_start(wt[:], w[:, g * NB * 4 * S:(g + 1) * NB * 4 * S])
                ot = opool.tile([B, NB * 2 * S], mybir.dt.float32)
                for i in range(NB):
                    a = g * NB + i
                    pt = ps.tile([B, 2 * S], mybir.dt.float32)
                    nc.tensor.matmul(
                        pt[:],
                        xr_t[:, a * B:(a + 1) * B],
                        wt[:, i * 4 * S:i * 4 * S + 2 * S],
                        start=True, stop=False,
                    )
                    nc.tensor.matmul(
                        pt[:],
                        xi_t[:, a * B:(a + 1) * B],
                        wt[:, i * 4 * S + 2 * S:(i + 1) * 4 * S],
                        start=False, stop=True,
                    )
                    if i % 2 == 0:
                        nc.vector.tensor_copy(ot[:, i * 2 * S:(i + 1) * 2 * S], pt[:])
                    else:
                        nc.scalar.copy(ot[:, i * 2 * S:(i + 1) * 2 * S], pt[:])
                nc.sync.dma_start(y[:, g * NB * 2 * S:(g + 1) * NB * 2 * S], ot[:])
    nc.compile()
    return nc


def kernel(x, hr1, hi1, hr2, hi2, perm_idx):
    from concourse.bass_utils import run_bass_kernel_spmd

    if "nc" not in _NC_CACHE:
        _NC_CACHE["nc"] = _build_nc()
    nc = _NC_CACHE["nc"]

    x = np.asarray(x, dtype=np.float32)
    perm_idx = np.asarray(perm_idx)
    # host-side permutation gather + regroup into M blocks of size S
    xp = x[:, :, perm_idx, :].reshape(B, 2, M, S)

    in_maps = []
    for c in range(NCORES):
        a0 = c * MLOC
        # [B, MLOC, S] -> [S(j), MLOC, B] -> [S, MLOC*B]
        xre = np.ascontiguousarray(
            np.transpose(xp[:, 0, a0:a0 + MLOC, :], (2, 1, 0))
        ).reshape(S, MLOC * B)
        xim = np.ascontiguousarray(
            np.transpose(xp[:, 1, a0:a0 + MLOC, :], (2, 1, 0))
        ).reshape(S, MLOC * B)
        # per block a: cols [hr1[a] | hi2[a] | hi1[a] | hr2[a]]  ([a, j, 4S])
        wc = np.concatenate(
            [hr1[a0:a0 + MLOC], hi2[a0:a0 + MLOC],
             hi1[a0:a0 + MLOC], hr2[a0:a0 + MLOC]], axis=2,
        )
        wc = np.ascontiguousarray(np.transpose(wc, (1, 0, 2))).reshape(S, MLOC * 4 * S)
        in_maps.append({"xr": xre, "xi": xim, "w": np.asarray(wc, dtype=np.float32)})

    trace = bool(os.environ.get("KERNEL_TRACE"))
    kwargs = {}
    if trace:
        kwargs["tmpdir"] = os.environ.get("KERNEL_TRACE_DIR") or None
    res = run_bass_kernel_spmd(nc, in_maps, core_ids=list(range(NCORES)), trace=trace, **kwargs)
    if trace and res.exec_time_ns is not None:
        print(f"HW exec time: {res.exec_time_ns} ns")
        _NC_CACHE["exec_time_ns"] = res.exec_time_ns
        _NC_CACHE["profile"] = res

    out = np.empty((B, 2, M, S), dtype=np.float32)
    for c in range(NCORES):
        a0 = c * MLOC
        yc = res.results[c]["y"].reshape(B, MLOC, 2, S)
        out[:, 0, a0:a0 + MLOC, :] = yc[:, :, 0, :]
        out[:, 1, a0:a0 + MLOC, :] = yc[:, :, 1, :]
    return out.reshape(B, 2, N, R)


# revision 4
# speedup vs baseline: 1.2871x; 1.2871x over previous
"""Block-diagonal complex matmul kernel for trn2 (8 NeuronCores).

Reference computation:
  xp = take(x, perm_idx, axis=-2).reshape(B, 2, M, S)
  y_re = xp_re @ hr1 + xp_im @ hi1   (per block a of M)
  y_im = xp_re @ hi2 + xp_im @ hr2
  out  = stack([y_re, y_im], 1).reshape(B, 2, N, R)

Sharding: block dim M=1024 split across 8 cores (128 blocks each).
Permutation gather + all layout shuffles happen host-side in numpy.

Per-core device kernel, per block a:
  psum[16, 256] = xT_re[:, a] .T @ [hr1[a] | hi2[a]]   (start)
                + xT_im[:, a] .T @ [hi1[a] | hr2[a]]   (stop)
  -> cols 0:128 = y_re[a], cols 128:256 = y_im[a]
"""

import os
import numpy as np

B = 16
N = 4096
R = 32
M = 1024   # blocks
S = 128    # block size (contract dim)
NCORES = 8
MLOC = M // NCORES   # 128 blocks per core
NB = 8               # blocks per weight DMA group (2 MiB per dma_start)
NGRP = MLOC // NB

_NC_CACHE = {}


def _build_nc():
    import concourse.bacc as bacc
    import concourse.bass as bass
    import concourse.mybir as mybir
    from concourse import tile

    mm_dt = mybir.dt.float32r
    nc = bacc.Bacc(None, target_bir_lowering=False)

    xr = nc.dram_tensor("xr", [S, MLOC * B], mm_dt, kind="ExternalInput")
    xi = nc.dram_tensor("xi", [S, MLOC * B], mm_dt, kind="ExternalInput")
    w = nc.dram_tensor("w", [S, MLOC * 4 * S], mm_dt, kind="ExternalInput")
    y = nc.dram_tensor("y", [B, MLOC * 2 * S], mybir.dt.float32, kind="ExternalOutput")

    with tile.TileContext(nc) as tc:
        with (
            tc.tile_pool(name="xp", bufs=1) as xpool,
            tc.tile_pool(name="wp", bufs=3) as wpool,
            tc.tile_pool(name="op", bufs=3) as opool,
            tc.tile_pool(name="ps", bufs=8, space=bass.MemorySpace.PSUM) as ps,
        ):
            xr_t = xpool.tile([S, MLOC * B], mm_dt)
            xi_t = xpool.tile([S, MLOC * B], mm_dt)
            nc.sync.dma_start(xr_t[:], xr[:])
            nc.sync.dma_start(xi_t[:], xi[:])
            for g in range(NGRP):
                wt = wpool.tile([S, NB * 4 * S], mm_dt)
                nc.sync.dma_start(wt[:], w[:, g * NB * 4 * S:(g + 1) * NB * 4 * S])
                ot = opool.tile([B, NB * 2 * S], mybir.dt.float32)
                for i in range(NB):
                    a = g * NB + i
                    pt = ps.tile([B, 2 * S], mybir.dt.float32)
                    nc.tensor.matmul(
                        pt[:],
                        xr_t[:, a * B:(a + 1) * B],
                        wt[:, i * 4 * S:i * 4 * S + 2 * S],
                        start=True, stop=False,
                    )
                    nc.tensor.matmul(
                        pt[:],
                        xi_t[:, a * B:(a + 1) * B],
                        wt[:, i * 4 * S + 2 * S:(i + 1) * 4 * S],
                        start=False, stop=True,
                    )
                    if i % 2 == 0:
                        nc.vector.tensor_copy(ot[:, i * 2 * S:(i + 1) * 2 * S], pt[:])
                    else:
                        nc.scalar.copy(ot[:, i * 2 * S:(i + 1) * 2 * S], pt[:])
                nc.sync.dma_start(y[:, g * NB * 2 * S:(g + 1) * NB * 2 * S], ot[:])
    nc.compile()
    return nc


def kernel(x, hr1, hi1, hr2, hi2, perm_idx):
    from concourse.bass_utils import run_bass_kernel_spmd

    if "nc" not in _NC_CACHE:
        _NC_CACHE["nc"] = _build_nc()
    nc = _NC_CACHE["nc"]

    x = np.asarray(x, dtype=np.float32)
    perm_idx = np.asarray(perm_idx)
    # host-side permutation gather + regroup into M blocks of size S
    xp = x[:, :, perm_idx, :].reshape(B, 2, M, S)

    in_maps = []
    for c in range(NCORES):
        a0 = c * MLOC
        # [B, MLOC, S] -> [S(j), MLOC, B] -> [S, MLOC*B]
        xre = np.ascontiguousarray(
            np.transpose(xp[:, 0, a0:a0 + MLOC, :], (2, 1, 0))
        ).reshape(S, MLOC * B)
        xim = np.ascontiguousarray(
            np.transpose(xp[:, 1, a0:a0 + MLOC, :], (2, 1, 0))
        ).reshape(S, MLOC * B)
        # per block a: cols [hr1[a] | hi2[a] | hi1[a] | hr2[a]]  ([a, j, 4S])
        wc = np.concatenate(
            [hr1[a0:a0 + MLOC], hi2[a0:a0 + MLOC],
             hi1[a0:a0 + MLOC], hr2[a0:a0 + MLOC]], axis=2,
        )
        wc = np.ascontiguousarray(np.transpose(wc, (1, 0, 2))).reshape(S, MLOC * 4 * S)
        in_maps.append({"xr": xre, "xi": xim, "w": np.asarray(wc, dtype=np.float32)})

    trace = bool(os.environ.get("KERNEL_TRACE"))
    kwargs = {}
    if trace:
        kwargs["tmpdir"] = os.environ.get("KERNEL_TRACE_DIR") or None
    res = run_bass_kernel_spmd(nc, in_maps, core_ids=list(range(NCORES)), trace=trace, **kwargs)
    if trace and res.exec_time_ns is not None:
        print(f"HW exec time: {res.exec_time_ns} ns")
        _NC_CACHE["exec_time_ns"] = res.exec_time_ns
        _NC_CACHE["profile"] = res

    out = np.empty((B, 2, M, S), dtype=np.float32)
    for c in range(NCORES):
        a0 = c * MLOC
        yc = res.results[c]["y"].reshape(B, MLOC, 2, S)
        out[:, 0, a0:a0 + MLOC, :] = yc[:, :, 0, :]
        out[:, 1, a0:a0 + MLOC, :] = yc[:, :, 1, :]
    return out.reshape(B, 2, N, R)
